# revision 1
# baseline (speedup 1.0000x reference)
"""MoE feed-forward kernel for 8 Trainium2 NeuronCores.

Strategy:
  - Router (tiny: x @ rW, top-2, softmax) runs on host in numpy.
  - Expert-parallel: core e owns routed expert e. Host gathers the tokens
    routed to expert e (padded to the global max capacity C), ships them
    pre-transposed as (D, C); the device runs gelu(x@W1+b1) @ W2 with the
    per-token gate weight folded in on-chip. Host scatter-adds the result.
  - Shared experts: sharded (expert s = core//4, hidden-quarter q = core%4).
    Each core computes its quarter of one shared expert over all tokens;
    host sums the 8 partials (0.5 mean factor folded into sW2 upload).
  - Matmuls run in float32r (fast fp32 mode, 1 cycle/row at N>=256).
"""

import sys
import types

import numpy as np

sys.path.insert(0, "/opt/trn_rl_repo")

import concourse.bass as bass  # noqa: E402
import concourse.mybir as mybir  # noqa: E402
import concourse.tile as tile  # noqa: E402
from concourse import bacc  # noqa: E402
from concourse.bass_utils import run_bass_kernel_spmd  # noqa: E402

F32 = mybir.dt.float32
F32R = mybir.dt.float32r
GELU = mybir.ActivationFunctionType.Gelu
ADD = mybir.AluOpType.add

D = 1024      # d_model
H = 4096      # expert hidden
HQ = 1024     # shared-expert hidden slice per core (H / 4)
T = 4096      # tokens (2 * 2048)
E = 8         # routed experts
TOP_K = 2
NCORES = 8


def _install_ntff_hook():
    """Shim for the missing antenv.axon_hooks so trace=True can profile."""
    try:
        import antenv
        if "antenv.axon_hooks" in sys.modules:
            return
        mod = types.ModuleType("antenv.axon_hooks")
        mod._hook = None
        mod.set_axon_ntff_profile_hook = lambda h: setattr(mod, "_hook", h)
        mod.get_axon_ntff_profile_hook = lambda: mod._hook
        sys.modules["antenv.axon_hooks"] = mod
        antenv.axon_hooks = mod
        sys.path.insert(0, "/root/.axon_site/trn_agent_boot")
        import trn_boot
        hook = trn_boot._ntff_profile_via_ctypes("/opt/axon/libaxon_pjrt.so")
        mod.set_axon_ntff_profile_hook(hook)
    except Exception:
        pass


def _split_cblocks(c):
    """Split C into token blocks sized {256,384,512} (multiples of 128) so the
    fp32r moving dim stays >=256; a lone 128 only if C == 128."""
    blocks = []
    rem = c
    start = 0
    while rem > 0:
        if rem >= 768:
            sz = 512
        elif rem == 640:
            sz = 384
        else:  # 128..512
            sz = rem
        blocks.append((start, sz))
        start += sz
        rem -= sz
    return blocks


_NC_CACHE = {}


def _build_nc(C):
    if C in _NC_CACHE:
        return _NC_CACHE[C]
    CR = C // 128
    cblocks = _split_cblocks(C)

    nc = bacc.Bacc("TRN2", target_bir_lowering=False, debug=False,
                   enable_asserts=True, num_devices=NCORES)

    xeT = nc.dram_tensor("xeT", (D, C), F32R, kind="ExternalInput")
    g_d = nc.dram_tensor("g", (CR, 128), F32, kind="ExternalInput")
    W1e = nc.dram_tensor("W1e", (D, H), F32R, kind="ExternalInput")
    W2e = nc.dram_tensor("W2e", (H, D), F32R, kind="ExternalInput")
    b1e = nc.dram_tensor("b1e", (H,), F32, kind="ExternalInput")
    xT = nc.dram_tensor("xT", (D, T), F32R, kind="ExternalInput")
    sW1q = nc.dram_tensor("sW1q", (D, HQ), F32R, kind="ExternalInput")
    sW2q = nc.dram_tensor("sW2q", (HQ, D), F32R, kind="ExternalInput")
    sb1q = nc.dram_tensor("sb1q", (HQ,), F32, kind="ExternalInput")
    yr = nc.dram_tensor("yr", (C, D), F32, kind="ExternalOutput")
    ys = nc.dram_tensor("ys", (T, D), F32, kind="ExternalOutput")

    with tile.TileContext(nc) as tc:
        # prefetch pool spans both phases: routed-phase x + first hidden-block
        # weights load during the shared phase so the transition has no stall
        with tc.tile_pool(name="rpre", bufs=1) as rpre:
          xe = rpre.tile([128, 8, C], F32R)
          w1p = rpre.tile([128, 8, 256], F32R)
          w2p = rpre.tile([128, 2, D], F32R)
          gt = rpre.tile([128, CR], F32)
          b1t = rpre.tile([128, 32], F32)
          xer = xeT.ap().rearrange("(a p) t -> p a t", p=128)
          W1r = W1e.ap().rearrange("(a p) h -> p a h", p=128)
          W2r = W2e.ap().rearrange("(a p) d -> p a d", p=128)

          # ---------------- phase S: shared-expert slice over all tokens ----
          with tc.tile_pool(name="swp", bufs=1) as swp, \
             tc.tile_pool(name="sxp", bufs=2) as sxp, \
             tc.tile_pool(name="shp", bufs=10) as shp, \
             tc.tile_pool(name="syp", bufs=3) as syp, \
             tc.tile_pool(name="sph", bufs=2, space="PSUM") as sph, \
             tc.tile_pool(name="spy", bufs=4, space="PSUM") as spy:
            sw1 = swp.tile([128, 8, HQ], F32R)
            sw2 = swp.tile([128, 8, D], F32R)
            sb1t = swp.tile([128, 8], F32)
            # Startup critical path: first GEMM chain needs all of sw1 + xs[0].
            # Each DMA queue tops out ~150GB/s, so split sw1 across two queues
            # and keep everything non-critical strictly BEHIND the critical
            # loads (queues have no priorities — order is the only lever).
            sw1r = sW1q.ap().rearrange("(a p) h -> p a h", p=128)
            nc.sync.dma_start(sw1[:, 0:4, :], sw1r[:, 0:4, :])
            nc.scalar.dma_start(sw1[:, 4:8, :], sw1r[:, 4:8, :])
            # sw2 needed when GEMM2 of block 0 starts (~28us in)
            nc.sync.dma_start(sw2[:], sW2q.ap().rearrange("(a p) d -> p a d", p=128)[:])
            nc.sync.dma_start(sb1t[:], sb1q.ap().rearrange("(a p) -> p a", p=128)[:])
            # routed-phase prefetch (rides far behind the shared-phase loads)
            nc.scalar.dma_start(xe[:], xer[:])
            nc.scalar.dma_start(w1p[:], W1r[:, :, 0:256])
            nc.sync.dma_start(w2p[:], W2r[:, 0:2, :])
            nc.sync.dma_start(gt[:], g_d.ap().rearrange("a p -> p a")[:])
            nc.sync.dma_start(b1t[:], b1e.ap().rearrange("(a p) -> p a", p=128)[:])
            xTr = xT.ap().rearrange("(a p) t -> p a t", p=128)
            ysr = ys.ap().rearrange("(a p) d -> p a d", p=128)
            for cb in range(T // 512):
                xs = sxp.tile([128, 8, 512], F32R, tag="xs")
                nc.gpsimd.dma_start(xs[:], xTr[:, :, cb * 512:(cb + 1) * 512])
                hts = []
                for h in range(8):
                    ph = sph.tile([128, 512], F32, tag="ph")
                    for d in range(8):
                        nc.tensor.matmul(ph[:], sw1[:, d, h * 128:(h + 1) * 128],
                                         xs[:, d, :], start=(d == 0), stop=(d == 7))
                    ht = shp.tile([128, 512], F32R, tag="ht")
                    nc.scalar.activation(ht[:], ph[:], GELU, bias=sb1t[:, h:h + 1])
                    hts.append(ht)
                for cs in range(4):
                    for dh in range(2):
                        py = spy.tile([128, 512], F32, tag="py")
                        for h in range(8):
                            nc.tensor.matmul(py[:], hts[h][:, cs * 128:(cs + 1) * 128],
                                             sw2[:, h, dh * 512:(dh + 1) * 512],
                                             start=(h == 0), stop=(h == 7))
                        yt = syp.tile([128, 512], F32, tag="yt")
                        nc.vector.tensor_copy(yt[:], py[:])
                        nc.gpsimd.dma_start(ysr[:, cb * 4 + cs, dh * 512:(dh + 1) * 512], yt[:])

          # ---------------- phase R: routed expert -------------------------
          NHB = 16  # hidden blocks of 256 (2 h-tiles each)
          with tc.tile_pool(name="rwp", bufs=2) as rwp, \
             tc.tile_pool(name="rhp", bufs=6) as rhp, \
             tc.tile_pool(name="ryp", bufs=1) as ryp, \
             tc.tile_pool(name="rgp", bufs=2) as rgp, \
             tc.tile_pool(name="rph", bufs=2, space="PSUM") as rph, \
             tc.tile_pool(name="rpy", bufs=4, space="PSUM") as rpy:
            y_acc = ryp.tile([128, CR, D], F32)
            yrr = yr.ap().rearrange("(a p) d -> p a d", p=128)
            for hb in range(NHB):
                if hb == 0:
                    w1, w2 = w1p, w2p  # prefetched during the shared phase
                else:
                    w1 = rwp.tile([128, 8, 256], F32R, tag="w1")
                    w2 = rwp.tile([128, 2, D], F32R, tag="w2")
                    nc.sync.dma_start(w1[:], W1r[:, :, hb * 256:(hb + 1) * 256])
                    nc.gpsimd.dma_start(w2[:], W2r[:, hb * 2:(hb + 1) * 2, :])
                for (c0, csz) in cblocks:
                    hts = []
                    for h in range(2):
                        ph = rph.tile([128, 512], F32, tag="ph")
                        for d in range(8):
                            nc.tensor.matmul(ph[:, :csz],
                                             w1[:, d, h * 128:(h + 1) * 128],
                                             xe[:, d, c0:c0 + csz],
                                             start=(d == 0), stop=(d == 7))
                        ht = rhp.tile([128, 512], F32R, tag="ht")
                        nc.scalar.activation(ht[:, :csz], ph[:, :csz], GELU,
                                             bias=b1t[:, hb * 2 + h:hb * 2 + h + 1])
                        hts.append(ht)
                    for cs in range(csz // 128):
                        crow = c0 // 128 + cs
                        for dh in range(2):
                            py = rpy.tile([128, 512], F32, tag="py")
                            for h in range(2):
                                nc.tensor.matmul(py[:],
                                                 hts[h][:, cs * 128:(cs + 1) * 128],
                                                 w2[:, h, dh * 512:(dh + 1) * 512],
                                                 start=(h == 0), stop=(h == 1))
                            dst = y_acc[:, crow, dh * 512:(dh + 1) * 512]
                            if hb == 0:
                                nc.vector.tensor_copy(dst, py[:])
                            else:
                                nc.vector.tensor_tensor(dst, dst, py[:], ADD)
                        if hb == NHB - 1:
                            # final hidden block: gate + store this row now so
                            # the epilogue overlaps the remaining matmuls
                            yg = rgp.tile([128, D], F32, tag="yg")
                            nc.scalar.mul(yg[:], y_acc[:, crow, :], gt[:, crow:crow + 1])
                            nc.gpsimd.dma_start(yrr[:, crow, :], yg[:])

    nc.compile()
    nc.finalize()
    _NC_CACHE[C] = nc
    return nc


def _route(xf, rW, rb):
    """Host router: replicates jax top_k (ties -> lower index) + softmax."""
    gates = xf @ rW + rb
    idx = np.argsort(-gates, axis=1, kind="stable")[:, :TOP_K]
    vals = np.take_along_axis(gates, idx, axis=1)
    ex = np.exp(vals - vals[:, :1])
    probs = (ex / ex.sum(axis=1, keepdims=True)).astype(np.float32)
    return idx, probs


def _run(inputs, trace=False):
    x = np.asarray(inputs["x"], dtype=np.float32)
    rW = np.asarray(inputs["rW"], dtype=np.float32)
    rb = np.asarray(inputs["rb"], dtype=np.float32)
    W1 = np.asarray(inputs["W1"], dtype=np.float32)
    b1 = np.asarray(inputs["b1"], dtype=np.float32)
    W2 = np.asarray(inputs["W2"], dtype=np.float32)
    b2 = np.asarray(inputs["b2"], dtype=np.float32)
    sW1 = np.asarray(inputs["sW1"], dtype=np.float32)
    sb1 = np.asarray(inputs["sb1"], dtype=np.float32)
    sW2 = np.asarray(inputs["sW2"], dtype=np.float32)
    sb2 = np.asarray(inputs["sb2"], dtype=np.float32)

    B, L, _ = x.shape
    xf = np.ascontiguousarray(x.reshape(-1, D))
    idx, probs = _route(xf, rW, rb)

    tok = []
    prb = []
    for e in range(E):
        sel = idx == e  # (T, K)
        rows = np.nonzero(sel.any(axis=1))[0]
        p = np.where(sel[rows, 0], probs[rows, 0], probs[rows, 1])
        tok.append(rows)
        prb.append(p.astype(np.float32))
    C = max(128, max((len(r) + 127) // 128 * 128 for r in tok))
    CR = C // 128

    nc = _build_nc(C)

    xT_full = np.ascontiguousarray(xf.T)
    in_maps = []
    for core in range(NCORES):
        s, q = core // 4, core % 4
        n_e = len(tok[core])
        xe = np.zeros((D, C), dtype=np.float32)
        xe[:, :n_e] = xf[tok[core]].T
        g = np.zeros((CR, 128), dtype=np.float32)
        g.reshape(-1)[:n_e] = prb[core]
        in_maps.append({
            "xeT": xe,
            "g": g,
            "W1e": np.ascontiguousarray(W1[core]),
            "W2e": np.ascontiguousarray(W2[core]),
            "b1e": np.ascontiguousarray(b1[core]),
            "xT": xT_full,
            "sW1q": np.ascontiguousarray(sW1[s][:, q * HQ:(q + 1) * HQ]),
            "sW2q": np.ascontiguousarray(0.5 * sW2[s][q * HQ:(q + 1) * HQ, :]),
            "sb1q": np.ascontiguousarray(sb1[s][q * HQ:(q + 1) * HQ]),
        })

    if trace:
        _install_ntff_hook()
    res = run_bass_kernel_spmd(nc, in_maps, list(range(NCORES)), trace=trace)

    out = np.zeros((T, D), dtype=np.float32)
    for core in range(NCORES):
        out += res.results[core]["ys"]
    out += 0.5 * (sb2[0] + sb2[1])[None, :]
    for e in range(E):
        n_e = len(tok[e])
        out[tok[e]] += res.results[e]["yr"][:n_e]
        out[tok[e]] += prb[e][:, None] * b2[e][None, :]
    return out.reshape(B, L, D).astype(np.float32), res


def kernel(**inputs):
    out, _ = _run(inputs, trace=False)
    return out



# revision 9
# speedup vs baseline: 1.1451x; 1.1451x over previous
"""MoE feed-forward kernel for 8 Trainium2 NeuronCores.

Strategy (v2, bf16):
  - Router (tiny: x @ rW, top-2, softmax) runs on host in numpy.
  - Expert-parallel: core e owns routed expert e. Host gathers the tokens
    routed to expert e (padded to the global max capacity C), ships them
    pre-transposed as (D, C) in bf16; the device runs gelu(x@W1+b1) @ W2
    with the per-token gate weight folded in on-chip. Host scatter-adds.
  - Shared experts: sharded (expert s = core//4, hidden-quarter q = core%4).
    Each core computes its quarter of one shared expert over all tokens;
    host sums the 8 bf16 partials (0.5 mean factor folded into sW2).
  - All matmul operands bf16 (halves SBUF read pressure vs fp32r and all
    DMA traffic; rel-err ~3.4e-3, well under the 2e-2 gate).
  - Both phases software-pipeline GEMM1/GEMM2 one block apart so the PE
    never waits on the activation engine.
  - Routed GEMM2 accumulates K=4096 in 6 standing PSUM banks per
    384-token group (3 tok rows x 2 d-halves) - no DVE adds at all.
  - W2 (8.4MB bf16) is fully SBUF-resident, prefetched in the shared
    phase; W1 streams per 128-hidden block on two queues.
"""

import sys
import types

import numpy as np
import ml_dtypes

sys.path.insert(0, "/opt/trn_rl_repo")

import concourse.bass as bass  # noqa: E402
import concourse.mybir as mybir  # noqa: E402
import concourse.tile as tile  # noqa: E402
from concourse import bacc  # noqa: E402
from concourse.bass_utils import run_bass_kernel_spmd  # noqa: E402

F32 = mybir.dt.float32
BF16 = mybir.dt.bfloat16
GELU = mybir.ActivationFunctionType.Gelu

D = 1024      # d_model
H = 4096      # expert hidden
HQ = 1024     # shared-expert hidden slice per core (H / 4)
T = 4096      # tokens (2 * 2048)
E = 8         # routed experts
TOP_K = 2
NCORES = 8
NHB = H // 128  # 32 hidden blocks in routed phase

BF = ml_dtypes.bfloat16


def _install_ntff_hook():
    """Shim for the missing antenv.axon_hooks so trace=True can profile."""
    try:
        import antenv
        if "antenv.axon_hooks" in sys.modules:
            return
        mod = types.ModuleType("antenv.axon_hooks")
        mod._hook = None
        mod.set_axon_ntff_profile_hook = lambda h: setattr(mod, "_hook", h)
        mod.get_axon_ntff_profile_hook = lambda: mod._hook
        sys.modules["antenv.axon_hooks"] = mod
        antenv.axon_hooks = mod
        sys.path.insert(0, "/root/.axon_site/trn_agent_boot")
        import trn_boot
        hook = trn_boot._ntff_profile_via_ctypes("/opt/axon/libaxon_pjrt.so")
        mod.set_axon_ntff_profile_hook(hook)
    except Exception:
        pass


def _tok_groups(CR):
    """Split CR token rows into groups of <=3 rows (6 PSUM banks each)."""
    groups = []
    r = 0
    while r < CR:
        n = min(3, CR - r)
        groups.append((r, n))
        r += n
    return groups


_NC_CACHE = {}


def _build_nc(C):
    if C in _NC_CACHE:
        return _NC_CACHE[C]
    CR = C // 128
    tgs = _tok_groups(CR)

    nc = bacc.Bacc("TRN2", target_bir_lowering=False, debug=False,
                   enable_asserts=True, num_devices=NCORES)

    xeT = nc.dram_tensor("xeT", (D, C), BF16, kind="ExternalInput")
    g_d = nc.dram_tensor("g", (CR, 128), F32, kind="ExternalInput")
    W1e = nc.dram_tensor("W1e", (D, H), BF16, kind="ExternalInput")
    W2e = nc.dram_tensor("W2e", (H, D), BF16, kind="ExternalInput")
    b1e = nc.dram_tensor("b1e", (H,), F32, kind="ExternalInput")
    xT = nc.dram_tensor("xT", (D, T), BF16, kind="ExternalInput")
    sW1q = nc.dram_tensor("sW1q", (D, HQ), BF16, kind="ExternalInput")
    sW2q = nc.dram_tensor("sW2q", (HQ, D), BF16, kind="ExternalInput")
    sb1q = nc.dram_tensor("sb1q", (HQ,), F32, kind="ExternalInput")
    yr = nc.dram_tensor("yr", (C, D), F32, kind="ExternalOutput")
    ys = nc.dram_tensor("ys", (T, D), BF16, kind="ExternalOutput")

    xer = xeT.ap().rearrange("(a p) t -> p a t", p=128)
    W1r = W1e.ap().rearrange("(a p) h -> p a h", p=128)
    W2r = W2e.ap().rearrange("(a p) d -> p a d", p=128)
    sw1r = sW1q.ap().rearrange("(a p) h -> p a h", p=128)
    sw2r = sW2q.ap().rearrange("(a p) d -> p a d", p=128)
    xTr = xT.ap().rearrange("(a p) t -> p a t", p=128)
    ysr = ys.ap().rearrange("(a p) d -> p a d", p=128)
    yrr = yr.ap().rearrange("(a p) d -> p a d", p=128)

    with tile.TileContext(nc) as tc:
        # long-lived pool: routed-phase tensors prefetched during phase S
        with tc.tile_pool(name="pre", bufs=1) as pre:
          w2f = pre.tile([128, NHB, D], BF16)     # full W2, 64KB/part
          xe = pre.tile([128, 8, C], BF16)        # routed tokens
          gt = pre.tile([128, CR], F32)
          b1t = pre.tile([128, NHB], F32)
          w1p = pre.tile([128, 2, 8, 128], BF16)  # W1 for hb=0,1 (prefetch)

          # ---------------- phase S: shared-expert slice over all tokens ----
          with tc.tile_pool(name="swp", bufs=1) as swp, \
             tc.tile_pool(name="sxp", bufs=3) as sxp, \
             tc.tile_pool(name="shp", bufs=18) as shp, \
             tc.tile_pool(name="syp", bufs=3) as syp, \
             tc.tile_pool(name="sph", bufs=2, space="PSUM") as sph, \
             tc.tile_pool(name="spy", bufs=4, space="PSUM") as spy:
            sw1 = swp.tile([128, 8, HQ], BF16)
            sw2 = swp.tile([128, 8, D], BF16)
            sb1t = swp.tile([128, 8], F32)

            # Startup critical path: first GEMM chain needs sw1 + xs[0].
            # Only sync/scalar/gpsimd can initiate DMAs; xs[0] leads the
            # gpsimd queue, sw1 splits across sync+scalar (1MB each). Bulk
            # prefetches are deferred (emitted after block 0's activations
            # on the scalar stream) so they can't steal HBM bandwidth from
            # the critical startup loads.
            nc.sync.dma_start(sw1[:, :, 0:512], sw1r[:, :, 0:512])
            nc.scalar.dma_start(sw1[:, :, 512:1024], sw1r[:, :, 512:1024])
            nc.sync.dma_start(sb1t[:], sb1q.ap().rearrange("(a p) -> p a", p=128)[:])
            # sw2 needed when GEMM2 of block 0 starts (~40us in)
            nc.sync.dma_start(sw2[:], sw2r[:])
            nc.sync.dma_start(gt[:], g_d.ap().rearrange("a p -> p a")[:])
            nc.sync.dma_start(b1t[:], b1e.ap().rearrange("(a p) -> p a", p=128)[:])
            nc.sync.dma_start(w1p[:, 0, :, :], W1r[:, :, 0:128])
            nc.sync.dma_start(w1p[:, 1, :, :], W1r[:, :, 128:256])

            NB = T // 512
            xs_t = [None] * NB
            hts_t = [None] * NB

            def s_g1(cb):
                xs = sxp.tile([128, 8, 512], BF16, tag="xs")
                nc.gpsimd.dma_start(xs[:], xTr[:, :, cb * 512:(cb + 1) * 512])
                xs_t[cb] = xs
                hts = []
                for h in range(8):
                    ph = sph.tile([128, 512], F32, tag="ph")
                    for d in range(8):
                        nc.tensor.matmul(ph[:], sw1[:, d, h * 128:(h + 1) * 128],
                                         xs[:, d, :], start=(d == 0), stop=(d == 7))
                    ht = shp.tile([128, 512], BF16, tag="ht")
                    nc.scalar.activation(ht[:], ph[:], GELU, bias=sb1t[:, h:h + 1])
                    hts.append(ht)
                hts_t[cb] = hts

            def s_g2(cb):
                hts = hts_t[cb]
                for cs in range(4):
                    for dh in range(2):
                        py = spy.tile([128, 512], F32, tag="py")
                        for h in range(8):
                            nc.tensor.matmul(py[:], hts[h][:, cs * 128:(cs + 1) * 128],
                                             sw2[:, h, dh * 512:(dh + 1) * 512],
                                             start=(h == 0), stop=(h == 7))
                        yt = syp.tile([128, 512], BF16, tag="yt")
                        nc.vector.tensor_copy(yt[:], py[:])
                        nc.gpsimd.dma_start(ysr[:, cb * 4 + cs, dh * 512:(dh + 1) * 512], yt[:])

            # software pipeline: G2(cb) sits between G1(cb+1) and G1(cb+2)
            s_g1(0)
            # routed-phase bulk prefetch: these triggers sit after block 0's
            # activations in the scalar stream, so the transfers start only
            # ~25us in, once the startup-critical loads are done.
            nc.scalar.dma_start(xe[:], xer[:])
            nc.scalar.dma_start(w2f[:], W2r[:])
            for cb in range(1, NB):
                s_g1(cb)
                s_g2(cb - 1)
            s_g2(NB - 1)

          # ---------------- phase R: routed expert -------------------------
          with tc.tile_pool(name="rwp", bufs=4) as rwp, \
             tc.tile_pool(name="rhp", bufs=3) as rhp, \
             tc.tile_pool(name="rgp", bufs=3) as rgp, \
             tc.tile_pool(name="rph", bufs=2, space="PSUM") as rph, \
             tc.tile_pool(name="rac", bufs=1, space="PSUM") as rac:
            for gi, (r0, nr) in enumerate(tgs):
                c0, ct = r0 * 128, nr * 128
                accs = [rac.tile([128, 512], F32, tag=f"acc{i}", bufs=1,
                                 name=f"acc{i}") for i in range(2 * nr)]
                ht_prev = None

                def r_g2(hb, ht):
                    for tr in range(nr):
                        for dh in range(2):
                            acc = accs[tr * 2 + dh]
                            nc.tensor.matmul(acc[:], ht[:, tr * 128:(tr + 1) * 128],
                                             w2f[:, hb, dh * 512:(dh + 1) * 512],
                                             start=(hb == 0), stop=(hb == NHB - 1))
                            if hb == NHB - 1:
                                # gate + store now so the epilogue overlaps
                                yg = rgp.tile([128, 512], F32, tag="yg")
                                crow = r0 + tr
                                nc.vector.tensor_scalar_mul(
                                    yg[:], acc[:], gt[:, crow:crow + 1])
                                nc.gpsimd.dma_start(
                                    yrr[:, crow, dh * 512:(dh + 1) * 512], yg[:])

                for hb in range(NHB):
                    if gi == 0 and hb < 2:
                        w1 = w1p[:, hb, :, :]
                    else:
                        w1t = rwp.tile([128, 8, 128], BF16, tag="w1")
                        q = nc.sync if hb % 2 == 0 else nc.gpsimd
                        q.dma_start(w1t[:], W1r[:, :, hb * 128:(hb + 1) * 128])
                        w1 = w1t[:]
                    ph = rph.tile([128, 512], F32, tag="ph")
                    for d in range(8):
                        nc.tensor.matmul(ph[:, :ct], w1[:, d, :],
                                         xe[:, d, c0:c0 + ct],
                                         start=(d == 0), stop=(d == 7))
                    ht = rhp.tile([128, 512], BF16, tag="ht")
                    nc.scalar.activation(ht[:, :ct], ph[:, :ct], GELU,
                                         bias=b1t[:, hb:hb + 1])
                    if ht_prev is not None:
                        r_g2(hb - 1, ht_prev)
                    ht_prev = ht
                r_g2(NHB - 1, ht_prev)

    nc.compile()
    nc.finalize()
    _NC_CACHE[C] = nc
    return nc


def _route(xf, rW, rb):
    """Host router: replicates jax top_k (ties -> lower index) + softmax."""
    gates = xf @ rW + rb
    idx = np.argsort(-gates, axis=1, kind="stable")[:, :TOP_K]
    vals = np.take_along_axis(gates, idx, axis=1)
    ex = np.exp(vals - vals[:, :1])
    probs = (ex / ex.sum(axis=1, keepdims=True)).astype(np.float32)
    return idx, probs


def _run(inputs, trace=False):
    x = np.asarray(inputs["x"], dtype=np.float32)
    rW = np.asarray(inputs["rW"], dtype=np.float32)
    rb = np.asarray(inputs["rb"], dtype=np.float32)
    W1 = np.asarray(inputs["W1"], dtype=np.float32)
    b1 = np.asarray(inputs["b1"], dtype=np.float32)
    W2 = np.asarray(inputs["W2"], dtype=np.float32)
    b2 = np.asarray(inputs["b2"], dtype=np.float32)
    sW1 = np.asarray(inputs["sW1"], dtype=np.float32)
    sb1 = np.asarray(inputs["sb1"], dtype=np.float32)
    sW2 = np.asarray(inputs["sW2"], dtype=np.float32)
    sb2 = np.asarray(inputs["sb2"], dtype=np.float32)

    B, L, _ = x.shape
    xf = np.ascontiguousarray(x.reshape(-1, D))
    idx, probs = _route(xf, rW, rb)

    tok = []
    prb = []
    for e in range(E):
        sel = idx == e  # (T, K)
        rows = np.nonzero(sel.any(axis=1))[0]
        p = np.where(sel[rows, 0], probs[rows, 0], probs[rows, 1])
        tok.append(rows)
        prb.append(p.astype(np.float32))
    C = max(128, max((len(r) + 127) // 128 * 128 for r in tok))
    CR = C // 128

    nc = _build_nc(C)

    xT_full = np.ascontiguousarray(xf.T).astype(BF)
    in_maps = []
    for core in range(NCORES):
        s, q = core // 4, core % 4
        n_e = len(tok[core])
        xe = np.zeros((D, C), dtype=BF)
        xe[:, :n_e] = xf[tok[core]].T.astype(BF)
        g = np.zeros((CR, 128), dtype=np.float32)
        g.reshape(-1)[:n_e] = prb[core]
        in_maps.append({
            "xeT": xe,
            "g": g,
            "W1e": np.ascontiguousarray(W1[core]).astype(BF),
            "W2e": np.ascontiguousarray(W2[core]).astype(BF),
            "b1e": np.ascontiguousarray(b1[core]),
            "xT": xT_full,
            "sW1q": np.ascontiguousarray(sW1[s][:, q * HQ:(q + 1) * HQ]).astype(BF),
            "sW2q": np.ascontiguousarray(0.5 * sW2[s][q * HQ:(q + 1) * HQ, :]).astype(BF),
            "sb1q": np.ascontiguousarray(sb1[s][q * HQ:(q + 1) * HQ]),
        })

    if trace:
        _install_ntff_hook()
    res = run_bass_kernel_spmd(nc, in_maps, list(range(NCORES)), trace=trace)

    out = np.zeros((T, D), dtype=np.float32)
    for core in range(NCORES):
        out += res.results[core]["ys"].astype(np.float32)
    out += 0.5 * (sb2[0] + sb2[1])[None, :]
    for e in range(E):
        n_e = len(tok[e])
        out[tok[e]] += res.results[e]["yr"][:n_e]
        out[tok[e]] += prb[e][:, None] * b2[e][None, :]
    return out.reshape(B, L, D).astype(np.float32), res


def kernel(**inputs):
    out, _ = _run(inputs, trace=False)
    return out
